# revision 10
# baseline (speedup 1.0000x reference)
"""CAM (channel attention) module kernel for Trainium2, 8-core data-parallel.

Reference computation (per sample, C=512, HW=4096):
    v = x.reshape(C, HW)
    E = v @ v.T                                  # (C, C)
    att = softmax(rowmax(E) - E, axis=-1)        # == softmax(-E) stabilized at rowmin
    o = att @ v                                  # (C, HW)
    o = softmax(o, axis=-1)
    out = x + gamma * o
Sharding: data-parallel over batch B=16 -> 2 samples per NeuronCore.

v4 design:
- x -> fp8 (vb, mm2's rhs) is a DVE cast, not a cast-DMA; DMA moves only
  the 16.8 MB load + 16.8 MB store per core (the HBM/fabric floor).
- v^T via *fp8* PE transposes of vb (107 ns vs 270 ns per block measured):
  step-2 [P, 2, C, 2] fp8 psum (one bank, a hardware requirement for fp8
  transpose outputs), one [128, 1024] strided fp8 eviction per pair.
- Both matmuls fp8 DoubleRow with fp32 PSUM (unchanged numerics).
- Softmax 1 skips normalization: 1/Z1 folded into the second exp's scale;
  both exps use ACT's fused row-sum accumulator.
- final out = x + (gamma/Z2)*exp on DVE reads the exact fp32 x tiles, so
  for gamma == 0 the output equals x bit-exactly.
- Schedule: strictly phased so the PE streams continuously (HAM clock-gate
  re-throttles the PE to 1.2 GHz after ~3.4 us idle): front(0) -> mm1
  pass2 -> attT(0) -> mm2(0) -> front(1) -> pass2 -> attT(1) -> mm2(1).
  Evictions go to whichever elementwise engine is idle in that phase:
  sample-N vT evictions ride ACT (idle during fronts), sample-1 casts ride
  DVE during mm2(0)'s exp2 window, STT(0) drains on DVE during front(1).
"""

import sys

if "/opt/trn_rl_repo" not in sys.path:
    sys.path.insert(0, "/opt/trn_rl_repo")

from contextlib import ExitStack

import numpy as np

P = 128
C = 512
HW = 4096
HHW = HW // 2  # 2048: half-width x tiles
QW = HW // 4  # 1024: quarter stripes
S = 2  # samples per core
CB = C // P  # 4 channel blocks
NB = HW // P  # 32 spatial blocks
NT = NB // 2  # 16 transpose pairs / DoubleRow k-pairs for matmul 1
NJ = HW // 1024  # 4 psum chunks (2 banks each) for the second matmul
N_CORES = 8

_NC = None


def _build_nc():
    import concourse.bacc as bacc
    import concourse.mybir as mybir
    import concourse.tile as tile
    from concourse.masks import make_identity

    f32 = mybir.dt.float32
    bf16 = mybir.dt.bfloat16
    fp8 = mybir.dt.float8e4
    AF = mybir.ActivationFunctionType
    ALU = mybir.AluOpType
    AX = mybir.AxisListType
    DR = mybir.MatmulPerfMode.DoubleRow

    nc = bacc.Bacc(
        "TRN2",
        target_bir_lowering=False,
        debug=False,
        num_devices=N_CORES,
        num_swdge_queues=2,
    )
    x = nc.dram_tensor("x", (S, C, HW), f32, kind="ExternalInput").ap()
    gamma = nc.dram_tensor("gamma", (1,), f32, kind="ExternalInput").ap()
    out = nc.dram_tensor("out", (S, C, HW), f32, kind="ExternalOutput").ap()

    with tile.TileContext(nc) as tc, ExitStack() as ctx:
        const = ctx.enter_context(tc.tile_pool(name="const", bufs=1))
        ident8 = const.tile([P, P], fp8)
        make_identity(nc, ident8)
        identb = const.tile([P, P], bf16)
        make_identity(nc, identb)
        gamma_sb = const.tile([P, 1], f32)
        nc.sync.dma_start(out=gamma_sb, in_=gamma.to_broadcast((P, 1)))

        xf_pool = ctx.enter_context(tc.tile_pool(name="xf_pool", bufs=14))
        vb_pool = ctx.enter_context(tc.tile_pool(name="vb_pool", bufs=4))
        vt_pool = ctx.enter_context(tc.tile_pool(name="vt_pool", bufs=NT))
        att_pool = ctx.enter_context(tc.tile_pool(name="att_pool", bufs=CB))
        attT_pool = ctx.enter_context(tc.tile_pool(name="attT_pool", bufs=2))
        exp_pool = ctx.enter_context(tc.tile_pool(name="exp_pool", bufs=3))
        small = ctx.enter_context(tc.tile_pool(name="small", bufs=12))
        r1_pool = ctx.enter_context(tc.tile_pool(name="r1_pool", bufs=10))
        psum_te = ctx.enter_context(tc.tile_pool(name="psum_te", bufs=4, space="PSUM"))
        psum_o = ctx.enter_context(tc.tile_pool(name="psum_o", bufs=2, space="PSUM"))

        # per-sample state
        xh = [[[None, None] for _ in range(CB)] for _ in range(S)]
        vb2 = [[None] * (CB // 2) for _ in range(S)]
        vT2 = [[None] * NT for _ in range(S)]
        att8 = [[None] * CB for _ in range(S)]
        r1s = [[None] * CB for _ in range(S)]
        attT2 = [[None] * (CB // 2) for _ in range(S)]

        def loads(s, quarters):
            # quarters=True: q0 stripes of the h=0 tiles first so the
            # bootstrap casts + first transposes start early; everything
            # else at half-width to keep the descriptor count down.
            # Sample 1 loads ride the GpSimd SWDGE queue so the two
            # samples' descriptor issue runs in parallel.
            eng = nc.gpsimd if s == 1 else nc.sync
            for h in range(2):
                for i in range(CB):
                    xh[s][i][h] = xf_pool.tile(
                        [P, HHW], f32, tag="xf", name=f"xf_{s}_{i}_{h}"
                    )
                if quarters and h == 0:
                    for q in range(2):
                        for i in range(CB):
                            eng.dma_start(
                                out=xh[s][i][h][:, q * QW : (q + 1) * QW],
                                in_=x[
                                    s,
                                    i * P : (i + 1) * P,
                                    h * HHW + q * QW : h * HHW + (q + 1) * QW,
                                ],
                            )
                else:
                    for i in range(CB):
                        eng.dma_start(
                            out=xh[s][i][h],
                            in_=x[s, i * P : (i + 1) * P, h * HHW : (h + 1) * HHW],
                        )

        def vb_alloc(s):
            for u in range(CB // 2):
                vb2[s][u] = vb_pool.tile([P, 2, HW], fp8, tag="vb", name=f"vb2_{s}_{u}")

        def vb_cast(s, h, q=None):
            # DVE f32 -> fp8 cast of one (h[, quarter]) stripe across all
            # channel blocks; feeds both the transposes and mm2's rhs.
            for i in range(CB):
                u, ko = divmod(i, 2)
                if q is None:
                    dst = vb2[s][u][:, ko, h * HHW : (h + 1) * HHW]
                    src = xh[s][i][h]
                else:
                    dst = vb2[s][u][:, ko, h * HHW + q * QW : h * HHW + (q + 1) * QW]
                    src = xh[s][i][h][:, q * QW : (q + 1) * QW]
                nc.vector.tensor_copy(dst, src)

        def v_transpose(s, t, evict):
            # vT pair t (n-part, c-free) fp8: fp8 PE transposes of vb blocks
            # into a step-2 [P, 2, C, 2] fp8 psum (one bank), one [128, 1024]
            # strided eviction (exact fp8 move) per pair.
            pt = psum_te.tile([P, 2, C, 2], fp8, tag="te", name=f"ptv_{s}_{t}")
            for ko in range(2):
                k = 2 * t + ko
                for i in range(CB):
                    nc.tensor.transpose(
                        pt[:, ko, i * P : (i + 1) * P, 0],
                        vb2[s][i // 2][:, i % 2, k * P : (k + 1) * P],
                        ident8,
                    )
            vt_ = vt_pool.tile([P, 2, C, 1], bf16, tag="vt", name=f"vT2_{s}_{t}")
            if evict == "scalar":
                nc.scalar.copy(vt_, pt.bitcast(bf16))
            else:
                nc.vector.tensor_copy(vt_, pt.bitcast(bf16))
            vT2[s][t] = vt_.bitcast(fp8)  # [P, 2, C, 2], data at [..., 0]

        def softmax1_tail(s, i, E):
            m = small.tile([P, 1], f32, tag="sm", name=f"m_{s}_{i}")
            nc.vector.tensor_reduce(m, E, axis=AX.X, op=ALU.min)
            a = att_pool.tile([P, C], bf16, tag="att", name=f"att_{s}_{i}")
            z1 = small.tile([P, 1], f32, tag="sm", name=f"z1_{s}_{i}")
            nc.scalar.activation(a, E, AF.Exp, bias=m, scale=-1.0, accum_out=z1)
            r1 = r1_pool.tile([P, 1], f32, tag="r1", name=f"r1_{s}_{i}")
            nc.vector.reciprocal(r1, z1)
            att8[s][i] = a
            r1s[s][i] = r1

        def mm1_block(s, i, E, t):
            nc.tensor.matmul(
                E,
                lhsT=vT2[s][t][:, :, i * P : (i + 1) * P, 0],
                rhs=vT2[s][t][:, :, :, 0],
                perf_mode=DR,
                start=(t == 0),
                stop=(t == NT - 1),
            )

        def front(s, evict, cast_hook=None):
            # transposes interleaved with mm1 row-blocks 0,1
            E01 = [
                psum_te.tile([P, C], f32, tag="te", name=f"E_{s}_{i}")
                for i in range(2)
            ]
            for t in range(NT):
                if cast_hook:
                    cast_hook(t)
                ev = evict if evict != "alternate" else ("vector" if t % 4 == 3 else "scalar")
                v_transpose(s, t, ev)
                for i in range(2):
                    mm1_block(s, i, E01[i], t)
            for i in range(2):
                softmax1_tail(s, i, E01[i])

        def mm1_pass2(s):
            E23 = [
                psum_te.tile([P, C], f32, tag="te", name=f"E_{s}_{i}")
                for i in range(2, CB)
            ]
            for t in range(NT):
                for i in range(2, CB):
                    mm1_block(s, i, E23[i - 2], t)
            for i in range(2, CB):
                softmax1_tail(s, i, E23[i - 2])

        def att_transposes(s, evict):
            # attT pairs (col-part, row-free) fp8 via bf16 PE transpose;
            # packed [P, 2, C] bf16 psum (one bank), one eviction per u.
            for u in range(CB // 2):
                pt = psum_te.tile([P, 2, C], bf16, tag="te", name=f"pta_{s}_{u}")
                for ko in range(2):
                    j = 2 * u + ko
                    for i in range(CB):
                        nc.tensor.transpose(
                            pt[:, ko, i * P : (i + 1) * P],
                            att8[s][i][:, j * P : (j + 1) * P],
                            identb,
                        )
                st = attT_pool.tile([P, 2, C], fp8, tag="attT", name=f"attT2_{s}_{u}")
                if evict == "scalar":
                    nc.scalar.copy(st, pt)
                else:
                    nc.vector.tensor_copy(st, pt)
                attT2[s][u] = st

        def mm2_final(s, i, store_engines=("sync", "sync")):
            # o = att @ v (DoubleRow), softmax over HW (with 1/Z1 folded into
            # the exp scale), then out = x + (gamma/Z2)*exp and store.
            er = exp_pool.tile([P, HW], bf16, tag="er", name=f"er_{s}_{i}")
            z2p = small.tile([P, NJ], f32, tag="z2p", name=f"z2p_{s}_{i}")
            for nj in range(NJ):
                o2 = psum_o.tile([P, 1024], f32, tag="o2", name=f"o2_{s}_{i}_{nj}")
                for hh in range(2):
                    sl = slice(nj * 1024 + hh * 512, nj * 1024 + (hh + 1) * 512)
                    for u in range(CB // 2):
                        nc.tensor.matmul(
                            o2[:, hh * 512 : (hh + 1) * 512],
                            lhsT=attT2[s][u][:, :, i * P : (i + 1) * P],
                            rhs=vb2[s][u][:, :, sl],
                            perf_mode=DR,
                            start=(u == 0),
                            stop=(u == CB // 2 - 1),
                        )
                nc.scalar.activation(
                    er[:, nj * 1024 : (nj + 1) * 1024],
                    o2,
                    AF.Exp,
                    scale=r1s[s][i],
                    accum_out=z2p[:, nj : nj + 1],
                )
            z2 = small.tile([P, 1], f32, tag="sm", name=f"z2_{s}_{i}")
            nc.vector.reduce_sum(z2, z2p, axis=AX.X)
            r2 = small.tile([P, 1], f32, tag="sm", name=f"r2_{s}_{i}")
            nc.vector.reciprocal(r2, z2)
            gz = small.tile([P, 1], f32, tag="sm", name=f"gz_{s}_{i}")
            nc.vector.tensor_scalar_mul(gz, r2, gamma_sb)
            for h in range(2):
                xt = xh[s][i][h]
                nc.vector.scalar_tensor_tensor(
                    out=xt,
                    in0=er[:, h * HHW : (h + 1) * HHW],
                    scalar=gz,
                    in1=xt,
                    op0=ALU.mult,
                    op1=ALU.add,
                )
                eng = nc.gpsimd if store_engines[h] == "gpsimd" else nc.sync
                eng.dma_start(
                    out=out[s, i * P : (i + 1) * P, h * HHW : (h + 1) * HHW],
                    in_=xt,
                )

        # ---- software pipeline across the two samples ----
        loads(0, quarters=True)
        loads(1, quarters=False)
        vb_alloc(0)
        vb_cast(0, 0, 0)  # bootstrap: h0 quarters first
        vb_cast(0, 0, 1)

        def cast0_hook(t):
            if t == 6:
                vb_cast(0, 1, 0)
            if t == 10:
                vb_cast(0, 1, 1)

        front(0, evict="alternate", cast_hook=cast0_hook)
        mm1_pass2(0)
        att_transposes(0, evict="vector")
        # sample-1 casts on DVE ahead of sample-0's STT blocks
        vb_alloc(1)
        vb_cast(1, 0)
        vb_cast(1, 1)
        mm2_final(0, 0, store_engines=("gpsimd", "sync"))
        mm2_final(0, 1, store_engines=("sync", "sync"))
        mm2_final(0, 2, store_engines=("gpsimd", "sync"))
        mm2_final(0, 3, store_engines=("sync", "sync"))
        # sample-1 front: ACT free again for evictions; DVE drains STT(0)
        front(1, evict="alternate")
        mm1_pass2(1)
        att_transposes(1, evict="vector")
        for i in range(CB):
            mm2_final(1, i, store_engines=("gpsimd", "sync") if i == 0 else ("sync", "sync"))

    nc.compile()
    return nc


def get_nc():
    global _NC
    if _NC is None:
        _NC = _build_nc()
    return _NC


def kernel(x: np.ndarray, gamma: np.ndarray) -> np.ndarray:
    from concourse.bass_utils import run_bass_kernel_spmd

    B, Cx, H, W = x.shape
    assert (B, Cx, H * W) == (16, C, HW), (B, Cx, H, W)
    nc = get_nc()
    xs = np.ascontiguousarray(np.asarray(x, dtype=np.float32)).reshape(B, Cx, H * W)
    g = np.ascontiguousarray(np.asarray(gamma, dtype=np.float32)).reshape(1)
    in_maps = [{"x": xs[S * c : S * (c + 1)], "gamma": g} for c in range(N_CORES)]
    res = run_bass_kernel_spmd(nc, in_maps, core_ids=list(range(N_CORES)))
    out = np.concatenate([res.results[c]["out"] for c in range(N_CORES)], axis=0)
    return out.reshape(B, Cx, H, W).astype(np.float32)


# revision 11
# speedup vs baseline: 1.2287x; 1.2287x over previous
"""CAM (channel attention) module kernel for Trainium2, 8-core data-parallel.

Reference computation (per sample, C=512, HW=4096):
    v = x.reshape(C, HW)
    E = v @ v.T                                  # (C, C)
    att = softmax(rowmax(E) - E, axis=-1)        # == softmax(-E) stabilized at rowmin
    o = att @ v                                  # (C, HW)
    o = softmax(o, axis=-1)
    out = x + gamma * o
Sharding: data-parallel over batch B=16 -> 2 samples per NeuronCore.

v4 design:
- x -> fp8 (vb, mm2's rhs) is a DVE cast, not a cast-DMA; DMA moves only
  the 16.8 MB load + 16.8 MB store per core (the HBM/fabric floor).
- v^T via *fp8* PE transposes of vb (107 ns vs 270 ns per block measured):
  step-2 [P, 2, C, 2] fp8 psum (one bank, a hardware requirement for fp8
  transpose outputs), one [128, 1024] strided fp8 eviction per pair.
- Both matmuls fp8 DoubleRow with fp32 PSUM (unchanged numerics).
- Softmax 1 skips normalization: 1/Z1 folded into the second exp's scale;
  both exps use ACT's fused row-sum accumulator.
- final out = x + (gamma/Z2)*exp on DVE reads the exact fp32 x tiles, so
  for gamma == 0 the output equals x bit-exactly.
- Schedule: strictly phased so the PE streams continuously (HAM clock-gate
  re-throttles the PE to 1.2 GHz after ~3.4 us idle): front(0) -> mm1
  pass2 -> attT(0) -> mm2(0) -> front(1) -> pass2 -> attT(1) -> mm2(1).
  Evictions go to whichever elementwise engine is idle in that phase:
  sample-N vT evictions ride ACT (idle during fronts), sample-1 casts ride
  DVE during mm2(0)'s exp2 window, STT(0) drains on DVE during front(1).
"""

import sys

if "/opt/trn_rl_repo" not in sys.path:
    sys.path.insert(0, "/opt/trn_rl_repo")

from contextlib import ExitStack

import numpy as np

P = 128
C = 512
HW = 4096
HHW = HW // 2  # 2048: half-width x tiles
QW = HW // 4  # 1024: quarter stripes
S = 2  # samples per core
CB = C // P  # 4 channel blocks
NB = HW // P  # 32 spatial blocks
NT = NB // 2  # 16 transpose pairs / DoubleRow k-pairs for matmul 1
NJ = HW // 1024  # 4 psum chunks (2 banks each) for the second matmul
N_CORES = 8

_NC = None


def _build_nc():
    import concourse.bacc as bacc
    import concourse.mybir as mybir
    import concourse.tile as tile
    from concourse.masks import make_identity

    f32 = mybir.dt.float32
    bf16 = mybir.dt.bfloat16
    fp8 = mybir.dt.float8e4
    AF = mybir.ActivationFunctionType
    ALU = mybir.AluOpType
    AX = mybir.AxisListType
    DR = mybir.MatmulPerfMode.DoubleRow

    nc = bacc.Bacc(
        "TRN2",
        target_bir_lowering=False,
        debug=False,
        num_devices=N_CORES,
        num_swdge_queues=2,
    )
    x = nc.dram_tensor("x", (S, C, HW), f32, kind="ExternalInput").ap()
    gamma = nc.dram_tensor("gamma", (1,), f32, kind="ExternalInput").ap()
    out = nc.dram_tensor("out", (S, C, HW), f32, kind="ExternalOutput").ap()

    with tile.TileContext(nc) as tc, ExitStack() as ctx:
        const = ctx.enter_context(tc.tile_pool(name="const", bufs=1))
        ident8 = const.tile([P, P], fp8)
        make_identity(nc, ident8)
        identb = const.tile([P, P], bf16)
        make_identity(nc, identb)
        gamma_sb = const.tile([P, 1], f32)
        nc.sync.dma_start(out=gamma_sb, in_=gamma.to_broadcast((P, 1)))

        xf_pool = ctx.enter_context(tc.tile_pool(name="xf_pool", bufs=14))
        vb_pool = ctx.enter_context(tc.tile_pool(name="vb_pool", bufs=4))
        vt_pool = ctx.enter_context(tc.tile_pool(name="vt_pool", bufs=NT))
        att_pool = ctx.enter_context(tc.tile_pool(name="att_pool", bufs=CB))
        attT_pool = ctx.enter_context(tc.tile_pool(name="attT_pool", bufs=2))
        exp_pool = ctx.enter_context(tc.tile_pool(name="exp_pool", bufs=3))
        small = ctx.enter_context(tc.tile_pool(name="small", bufs=12))
        r1_pool = ctx.enter_context(tc.tile_pool(name="r1_pool", bufs=10))
        psum_te = ctx.enter_context(tc.tile_pool(name="psum_te", bufs=4, space="PSUM"))
        psum_o = ctx.enter_context(tc.tile_pool(name="psum_o", bufs=2, space="PSUM"))

        # per-sample state
        xh = [[[None, None] for _ in range(CB)] for _ in range(S)]
        vb2 = [[None] * (CB // 2) for _ in range(S)]
        vT2 = [[None] * NT for _ in range(S)]
        att8 = [[None] * CB for _ in range(S)]
        r1s = [[None] * CB for _ in range(S)]
        attT2 = [[None] * (CB // 2) for _ in range(S)]

        def loads(s, quarters):
            # quarters=True: q0 stripes of the h=0 tiles first so the
            # bootstrap casts + first transposes start early; everything
            # else at half-width to keep the descriptor count down.
            # Sample 1 loads ride the GpSimd SWDGE queue so the two
            # samples' descriptor issue runs in parallel.
            eng = nc.sync
            for h in range(2):
                for i in range(CB):
                    xh[s][i][h] = xf_pool.tile(
                        [P, HHW], f32, tag="xf", name=f"xf_{s}_{i}_{h}"
                    )
                if quarters and h == 0:
                    for q in range(2):
                        for i in range(CB):
                            eng.dma_start(
                                out=xh[s][i][h][:, q * QW : (q + 1) * QW],
                                in_=x[
                                    s,
                                    i * P : (i + 1) * P,
                                    h * HHW + q * QW : h * HHW + (q + 1) * QW,
                                ],
                            )
                else:
                    for i in range(CB):
                        eng.dma_start(
                            out=xh[s][i][h],
                            in_=x[s, i * P : (i + 1) * P, h * HHW : (h + 1) * HHW],
                        )

        def vb_alloc(s):
            for u in range(CB // 2):
                vb2[s][u] = vb_pool.tile([P, 2, HW], fp8, tag="vb", name=f"vb2_{s}_{u}")

        def vb_cast(s, h, q=None):
            # DVE f32 -> fp8 cast of one (h[, quarter]) stripe across all
            # channel blocks; feeds both the transposes and mm2's rhs.
            for i in range(CB):
                u, ko = divmod(i, 2)
                if q is None:
                    dst = vb2[s][u][:, ko, h * HHW : (h + 1) * HHW]
                    src = xh[s][i][h]
                else:
                    dst = vb2[s][u][:, ko, h * HHW + q * QW : h * HHW + (q + 1) * QW]
                    src = xh[s][i][h][:, q * QW : (q + 1) * QW]
                nc.vector.tensor_copy(dst, src)

        def v_transpose(s, t, evict):
            # vT pair t (n-part, c-free) fp8: fp8 PE transposes of vb blocks
            # into a step-2 [P, 2, C, 2] fp8 psum (one bank), one [128, 1024]
            # strided eviction (exact fp8 move) per pair.
            pt = psum_te.tile([P, 2, C, 2], fp8, tag="te", name=f"ptv_{s}_{t}")
            for ko in range(2):
                k = 2 * t + ko
                for i in range(CB):
                    nc.tensor.transpose(
                        pt[:, ko, i * P : (i + 1) * P, 0],
                        vb2[s][i // 2][:, i % 2, k * P : (k + 1) * P],
                        ident8,
                    )
            vt_ = vt_pool.tile([P, 2, C, 1], bf16, tag="vt", name=f"vT2_{s}_{t}")
            if evict == "scalar":
                nc.scalar.copy(vt_, pt.bitcast(bf16))
            else:
                nc.vector.tensor_copy(vt_, pt.bitcast(bf16))
            vT2[s][t] = vt_.bitcast(fp8)  # [P, 2, C, 2], data at [..., 0]

        def softmax1_tail(s, i, E):
            m = small.tile([P, 1], f32, tag="sm", name=f"m_{s}_{i}")
            nc.vector.tensor_reduce(m, E, axis=AX.X, op=ALU.min)
            a = att_pool.tile([P, C], bf16, tag="att", name=f"att_{s}_{i}")
            z1 = small.tile([P, 1], f32, tag="sm", name=f"z1_{s}_{i}")
            nc.scalar.activation(a, E, AF.Exp, bias=m, scale=-1.0, accum_out=z1)
            r1 = r1_pool.tile([P, 1], f32, tag="r1", name=f"r1_{s}_{i}")
            nc.vector.reciprocal(r1, z1)
            att8[s][i] = a
            r1s[s][i] = r1

        def mm1_block(s, i, E, t):
            nc.tensor.matmul(
                E,
                lhsT=vT2[s][t][:, :, i * P : (i + 1) * P, 0],
                rhs=vT2[s][t][:, :, :, 0],
                perf_mode=DR,
                start=(t == 0),
                stop=(t == NT - 1),
            )

        def front(s, evict, cast_hook=None):
            # transposes interleaved with mm1 row-blocks 0,1
            E01 = [
                psum_te.tile([P, C], f32, tag="te", name=f"E_{s}_{i}")
                for i in range(2)
            ]
            for t in range(NT):
                if cast_hook:
                    cast_hook(t)
                ev = evict if evict != "alternate" else ("vector" if t % 4 == 3 else "scalar")
                v_transpose(s, t, ev)
                for i in range(2):
                    mm1_block(s, i, E01[i], t)
            for i in range(2):
                softmax1_tail(s, i, E01[i])

        def mm1_pass2(s):
            E23 = [
                psum_te.tile([P, C], f32, tag="te", name=f"E_{s}_{i}")
                for i in range(2, CB)
            ]
            for t in range(NT):
                for i in range(2, CB):
                    mm1_block(s, i, E23[i - 2], t)
            for i in range(2, CB):
                softmax1_tail(s, i, E23[i - 2])

        def att_transposes(s, evict):
            # attT pairs (col-part, row-free) fp8 via bf16 PE transpose;
            # packed [P, 2, C] bf16 psum (one bank), one eviction per u.
            for u in range(CB // 2):
                pt = psum_te.tile([P, 2, C], bf16, tag="te", name=f"pta_{s}_{u}")
                for ko in range(2):
                    j = 2 * u + ko
                    for i in range(CB):
                        nc.tensor.transpose(
                            pt[:, ko, i * P : (i + 1) * P],
                            att8[s][i][:, j * P : (j + 1) * P],
                            identb,
                        )
                st = attT_pool.tile([P, 2, C], fp8, tag="attT", name=f"attT2_{s}_{u}")
                if evict == "scalar":
                    nc.scalar.copy(st, pt)
                else:
                    nc.vector.tensor_copy(st, pt)
                attT2[s][u] = st

        def mm2_final(s, i, store_engines=("sync", "sync")):
            # o = att @ v (DoubleRow), softmax over HW (with 1/Z1 folded into
            # the exp scale), then out = x + (gamma/Z2)*exp and store.
            er = exp_pool.tile([P, HW], bf16, tag="er", name=f"er_{s}_{i}")
            z2p = small.tile([P, NJ], f32, tag="z2p", name=f"z2p_{s}_{i}")
            for nj in range(NJ):
                o2 = psum_o.tile([P, 1024], f32, tag="o2", name=f"o2_{s}_{i}_{nj}")
                for hh in range(2):
                    sl = slice(nj * 1024 + hh * 512, nj * 1024 + (hh + 1) * 512)
                    for u in range(CB // 2):
                        nc.tensor.matmul(
                            o2[:, hh * 512 : (hh + 1) * 512],
                            lhsT=attT2[s][u][:, :, i * P : (i + 1) * P],
                            rhs=vb2[s][u][:, :, sl],
                            perf_mode=DR,
                            start=(u == 0),
                            stop=(u == CB // 2 - 1),
                        )
                nc.scalar.activation(
                    er[:, nj * 1024 : (nj + 1) * 1024],
                    o2,
                    AF.Exp,
                    scale=r1s[s][i],
                    accum_out=z2p[:, nj : nj + 1],
                )
            z2 = small.tile([P, 1], f32, tag="sm", name=f"z2_{s}_{i}")
            nc.vector.reduce_sum(z2, z2p, axis=AX.X)
            r2 = small.tile([P, 1], f32, tag="sm", name=f"r2_{s}_{i}")
            nc.vector.reciprocal(r2, z2)
            gz = small.tile([P, 1], f32, tag="sm", name=f"gz_{s}_{i}")
            nc.vector.tensor_scalar_mul(gz, r2, gamma_sb)
            for h in range(2):
                xt = xh[s][i][h]
                nc.vector.scalar_tensor_tensor(
                    out=xt,
                    in0=er[:, h * HHW : (h + 1) * HHW],
                    scalar=gz,
                    in1=xt,
                    op0=ALU.mult,
                    op1=ALU.add,
                )
                eng = nc.gpsimd if store_engines[h] == "gpsimd" else nc.sync
                eng.dma_start(
                    out=out[s, i * P : (i + 1) * P, h * HHW : (h + 1) * HHW],
                    in_=xt,
                )

        # ---- software pipeline across the two samples ----
        loads(0, quarters=True)
        loads(1, quarters=False)
        vb_alloc(0)
        vb_cast(0, 0, 0)  # bootstrap: h0 quarters first
        vb_cast(0, 0, 1)

        def cast0_hook(t):
            if t == 6:
                vb_cast(0, 1, 0)
            if t == 10:
                vb_cast(0, 1, 1)

        front(0, evict="alternate", cast_hook=cast0_hook)
        mm1_pass2(0)
        att_transposes(0, evict="vector")
        # sample-1 casts on DVE ahead of sample-0's STT blocks
        vb_alloc(1)
        vb_cast(1, 0)
        vb_cast(1, 1)
        mm2_final(0, 0, store_engines=("gpsimd", "sync"))
        mm2_final(0, 1, store_engines=("sync", "sync"))
        mm2_final(0, 2, store_engines=("gpsimd", "sync"))
        mm2_final(0, 3, store_engines=("sync", "sync"))
        # sample-1 front: ACT free again for evictions; DVE drains STT(0)
        front(1, evict="alternate")
        mm1_pass2(1)
        att_transposes(1, evict="vector")
        for i in range(CB):
            mm2_final(1, i, store_engines=("gpsimd", "sync") if i == 0 else ("sync", "sync"))

    nc.compile()
    return nc


def get_nc():
    global _NC
    if _NC is None:
        _NC = _build_nc()
    return _NC


def kernel(x: np.ndarray, gamma: np.ndarray) -> np.ndarray:
    from concourse.bass_utils import run_bass_kernel_spmd

    B, Cx, H, W = x.shape
    assert (B, Cx, H * W) == (16, C, HW), (B, Cx, H, W)
    nc = get_nc()
    xs = np.ascontiguousarray(np.asarray(x, dtype=np.float32)).reshape(B, Cx, H * W)
    g = np.ascontiguousarray(np.asarray(gamma, dtype=np.float32)).reshape(1)
    in_maps = [{"x": xs[S * c : S * (c + 1)], "gamma": g} for c in range(N_CORES)]
    res = run_bass_kernel_spmd(nc, in_maps, core_ids=list(range(N_CORES)))
    out = np.concatenate([res.results[c]["out"] for c in range(N_CORES)], axis=0)
    return out.reshape(B, Cx, H, W).astype(np.float32)
